# revision 4
# baseline (speedup 1.0000x reference)
"""Trainium2 Bass kernel for nn_BilinearModule (16,256,64,64 bilinear pooling).

Math (per image):
  y   = relu(bn1(w1 @ x + b1))                       # (32, 4096)
  packed[t] = y[r_t] * y[c_t]  for 528 lower-tri pairs
  out = relu(bn2(w2 @ packed + b2))                  # (256, 4096)

Strategy (pure data parallel over batch, 2 images per core, 8 cores):
  - all matmul operands bf16 (x cast host-side, halves the input DMA);
    fp32 PSUM accumulation and fp32 BN math keep the error ~5e-3.
  - mm1 with M-replicated weights -> psum; fused BN1+ReLU on ACT -> yrep bf16
    (4 identical copies of the 32 channels across 128 partitions).
  - The 528 pair-products are covered by 17 channel rotations r=0..16:
    rotation r yields pairs {c, (c+r)%32} = diag r plus diag 32-r, all
    distinct (r=16 half-duplicated). Product tile j quadrant q needs
    rotation 4j+q. Rotated tiles come from DVE STREAM_SHUFFLE (~60ns each,
    vs ~200ns PE permutation matmuls in the previous version):
      ystep[32q+c] = y[(c+q)%32]   four [32,512] quadrant shuffles of y
      t_j = shuffle(ystep, rot 4j) one full-tile shuffle per j=1..4
    (stream_shuffle applies one 32-mask per 32-partition quadrant, so the
    stepped base tile makes a single mask produce 4 different rotations.)
  - Products: DVE tensor_mul (SBUF bf16 x SBUF bf16 -> bf16, ~330ns; PSUM
    reads on DVE are 5x slower, so everything stays in SBUF); one product
    per window goes to GpSimd to keep DVE under the PE window period.
  - mm2 = 5 K=128 bf16 chunks per output half with host-side permuted+
    zero-padded w2 (chunk 4 rows 32..128 are zero, killing the unused
    t4 quadrants); fused BN2+ReLU on ACT -> bf16, output DMA'd as bf16
    and upcast on the host (halves the output DMA).
  - 2-deep software pipeline: PE stream per window is mm1(w) | mm2(w-2),
    so BN1/shuffles/products have two windows of slack; DMA-free warmup
    matmuls (memset tile) open the HAM clock-gate during the preamble and
    bridge until the first x quarter lands (a PE gap here drops the PE
    p-state for ~17us, so the warmup count matters).
All weights are preprocessed host-side; pair order is folded into w2.
"""

import numpy as np

import concourse.bass as bass
import concourse.mybir as mybir
from concourse import tile
from concourse.bass_utils import run_bass_kernel_spmd

F32 = mybir.dt.float32
BF16 = mybir.dt.bfloat16
AF = mybir.ActivationFunctionType

N_CORES = 8
B, CIN, H, W = 16, 256, 64, 64
NPIX = H * W                     # 4096
IMG_PER_CORE = B // N_CORES      # 2
CMID = 32
COUT = 256
FB = 512                         # pixel window (psum-bank sized)
NWIN = NPIX // FB                # 8 windows per image
EPS = 1e-5
N_WARMUP = 28

# rotation sets per product tile (quadrant q of tile j uses ROTS[j][q])
ROTS = [[0, 1, 2, 3], [4, 5, 6, 7], [8, 9, 10, 11], [12, 13, 14, 15], [16, 0, 0, 0]]

_ctr = [0]


def _split_multi_waits(nc):
    """This container's walrus supports one sync-wait per instruction; split
    extras onto NOP carriers on the same engine."""
    for f in nc.m.functions:
        for blk in f.blocks:
            insts = blk.instructions
            if not any(
                i.sync_info is not None and len(i.sync_info.on_wait) > 1
                for i in insts
            ):
                continue
            new = []
            for inst in insts:
                si = inst.sync_info
                if si is not None and len(si.on_wait) > 1:
                    waits = list(si.on_wait)
                    for wcond in waits[:-1]:
                        _ctr[0] += 1
                        nop = mybir.InstNoOp(name=f"waitnop-{_ctr[0]}", ins=[], outs=[])
                        nop.engine = inst.engine
                        nop.sync_info = mybir.SyncInfo(on_wait=[wcond], on_update=[])
                        new.append(nop)
                    inst.sync_info = mybir.SyncInfo(
                        on_wait=[waits[-1]], on_update=list(si.on_update)
                    )
                new.append(inst)
            blk.instructions = new


def _host_weights(w1, b1, g1, be1, m1, v1, w2, b2, g2, be2, m2, v2):
    """Precompute device weight layouts on the host."""
    # mm1 lhsT, M-replicated: w1t[k, 32q+c] = w1[c, k]
    w1t = np.zeros((CIN, 128), np.float32)
    for q in range(4):
        w1t[:, 32 * q : 32 * q + 32] = w1.T
    inv1 = g1 / np.sqrt(v1 + EPS)
    bn1s = np.tile(inv1, 4).reshape(128, 1).astype(np.float32)
    bn1b = np.tile(b1 * inv1 + be1 - m1 * inv1, 4).reshape(128, 1).astype(np.float32)

    # w2 permuted into the 5x128 product-row order; duplicate slots zeroed.
    off = np.zeros(33, np.int64)
    for d in range(32):
        off[d + 1] = off[d] + (32 - d)
    assert off[32] == 528
    w2p = np.zeros((5 * 128, COUT), np.float32)
    used = np.zeros(528, bool)
    for j in range(5):
        for q in range(4):
            r = ROTS[j][q]
            if j == 4 and q > 0:
                continue  # spare quadrants: weights stay zero
            for c in range(32):
                if r == 16 and c >= 16:
                    continue  # duplicate half of rotation 16
                if c + r < 32:
                    d, b_lo = r, c
                else:
                    d, b_lo = 32 - r, c + r - 32
                t = off[d] + b_lo
                assert not used[t]
                used[t] = True
                w2p[128 * j + 32 * q + c, :] = w2[:, t]
    assert used.all()

    inv2 = g2 / np.sqrt(v2 + EPS)
    bn2s = inv2.reshape(2, 128).T.astype(np.float32).copy()   # [128, 2] col m
    bn2b = (b2 * inv2 + be2 - m2 * inv2).reshape(2, 128).T.astype(np.float32).copy()
    return w1t, bn1s, bn1b, w2p, bn2s, bn2b


def _build_nc():
    nc = bass.Bass()
    x_d = nc.declare_dram_parameter("x", [IMG_PER_CORE, CIN, NPIX], BF16, isOutput=False)
    w1t_d = nc.declare_dram_parameter("w1t", [CIN, 128], BF16, isOutput=False)
    bn1s_d = nc.declare_dram_parameter("bn1s", [128, 1], F32, isOutput=False)
    bn1b_d = nc.declare_dram_parameter("bn1b", [128, 1], F32, isOutput=False)
    w2p_d = nc.declare_dram_parameter("w2p", [5 * 128, COUT], BF16, isOutput=False)
    bn2s_d = nc.declare_dram_parameter("bn2s", [128, 2], F32, isOutput=False)
    bn2b_d = nc.declare_dram_parameter("bn2b", [128, 2], F32, isOutput=False)
    out_d = nc.declare_dram_parameter("out", [IMG_PER_CORE, COUT, NPIX], BF16, isOutput=True)

    with tile.TileContext(nc) as tc:
        with (
            tc.tile_pool(name="consts", bufs=1) as cpool,
            tc.tile_pool(name="xp", bufs=1) as xpool,
            tc.tile_pool(name="yp", bufs=4) as ypool,
            tc.tile_pool(name="sp", bufs=4) as stpool,
            tc.tile_pool(name="tp", bufs=12) as tpool,
            tc.tile_pool(name="pp", bufs=18) as ppool,
            tc.tile_pool(name="zp", bufs=6) as zpool,
            tc.tile_pool(name="psy", bufs=2, space="PSUM") as psum_y,
            tc.tile_pool(name="psz", bufs=4, space="PSUM") as psum_z,
        ):
            # -------- consts + x streaming (first window's data first) -----
            w1a = cpool.tile([128, 128], BF16, tag="w1a")
            w1b = cpool.tile([128, 128], BF16, tag="w1b")
            nc.sync.dma_start(w1a[:], w1t_d[0:128, :])
            nc.sync.dma_start(w1b[:], w1t_d[128:256, :])
            bn1s = cpool.tile([128, 1], F32, tag="bn1s")
            bn1b = cpool.tile([128, 1], F32, tag="bn1b")
            nc.sync.dma_start(bn1s[:], bn1s_d[:])
            nc.sync.dma_start(bn1b[:], bn1b_d[:])

            QRT = NPIX // 4
            xtiles = {}

            def load_x(img, h, split=1, eng=None):
                eng = eng or nc.sync
                xa = xpool.tile([128, QRT], BF16, tag=f"xa{img}{h}")
                xb = xpool.tile([128, QRT], BF16, tag=f"xb{img}{h}")
                step = QRT // split
                for p in range(split):
                    sl = slice(h * QRT + p * step, h * QRT + (p + 1) * step)
                    sd = slice(p * step, (p + 1) * step)
                    eng.dma_start(xa[:, sd], x_d[img, 0:128, sl])
                    eng.dma_start(xb[:, sd], x_d[img, 128:256, sl])
                xtiles[(img, h)] = (xa, xb)

            # first two quarters ride the ACT hardware DGE: issued during the
            # ACT preamble idle, in parallel with SP's const DMAs; the rest
            # stay on SP so they don't queue behind ACT compute.
            load_x(0, 0, split=2, eng=nc.scalar)
            load_x(0, 1, eng=nc.scalar)

            # Warm the PE clock gate (HAM) while the first x tiles stream in.
            wz = cpool.tile([128, 128], BF16, tag="warmz")
            nc.vector.memset(wz[:], 0.0)
            ps_warm = psum_y.tile([128, FB], F32, tag="psy")
            for _ in range(N_WARMUP):
                nc.tensor.matmul(
                    ps_warm[:, 0:128], wz[:], wz[:], start=True, stop=True
                )

            bn2s = cpool.tile([128, 2], F32, tag="bn2s")
            bn2b = cpool.tile([128, 2], F32, tag="bn2b")
            nc.sync.dma_start(bn2s[:], bn2s_d[:])
            nc.sync.dma_start(bn2b[:], bn2b_d[:])
            w2p_sb = cpool.tile([128, 5 * COUT], BF16, tag="w2p")
            for j in range(5):
                nc.sync.dma_start(
                    w2p_sb[:, j * COUT : (j + 1) * COUT],
                    w2p_d[j * 128 : (j + 1) * 128, :],
                )
            load_x(0, 2)
            load_x(0, 3)
            load_x(1, 0)
            load_x(1, 1)
            load_x(1, 2)
            load_x(1, 3)

            def stage_a(img, win):
                """mm1 + BN1 + stepped tile + rotations + products."""
                h, wl = divmod(win, NWIN // 4)
                s_loc = slice(wl * FB, (wl + 1) * FB)
                xa, xb = xtiles[(img, h)]
                ps_y = psum_y.tile([128, FB], F32, tag="psy")
                nc.tensor.matmul(ps_y[:], w1a[:], xa[:, s_loc], start=True, stop=False)
                nc.tensor.matmul(ps_y[:], w1b[:], xb[:, s_loc], start=False, stop=True)
                yrep = ypool.tile([128, FB], BF16, tag="yrep")
                nc.scalar.activation(
                    yrep[:], ps_y[:], AF.Relu, bias=bn1b[:, 0:1], scale=bn1s[:, 0:1]
                )
                # ystep[32q+c] = y[(c+q)%32]: 4 quadrant shuffles of y
                ystep = stpool.tile([128, FB], BF16, tag="ystep")
                for q in range(4):
                    mask = [(c + q) % 32 for c in range(32)]
                    nc.vector.stream_shuffle(
                        ystep[32 * q : 32 * q + 32, :], yrep[0:32, :], mask
                    )
                # t_j = shuffle(ystep, rot 4j): quadrant q holds rot 4j+q.
                # All shuffles issue before all muls: back-to-back write->read
                # of the same tile on DVE costs a ~550ns pipeline bubble.
                tiles = [ystep]
                for j in range(1, 5):
                    mask = [(c + 4 * j) % 32 for c in range(32)]
                    tj = tpool.tile([128, FB], BF16, tag="tj")
                    nc.vector.stream_shuffle(tj[:], ystep[:], mask)
                    tiles.append(tj)
                prods = []
                for j in range(5):
                    pj = ppool.tile([128, FB], BF16, tag="pj")
                    if j == 2:
                        # one product on GpSimd keeps DVE under the window period
                        nc.gpsimd.tensor_mul(pj[:], yrep[:], tiles[j][:])
                    else:
                        nc.vector.tensor_mul(pj[:], yrep[:], tiles[j][:])
                    prods.append(pj)
                return prods

            def stage_b_m(img, win, prods, m):
                """mm2 m-chunk + BN2 + store for one window."""
                s = slice(win * FB, (win + 1) * FB)
                J_ORDER = (0, 1, 3, 4, 2)  # GpSimd-produced chunk last
                ps_z = psum_z.tile([128, FB], F32, tag="psz")
                for idx, j in enumerate(J_ORDER):
                    nc.tensor.matmul(
                        ps_z[:],
                        w2p_sb[:, j * COUT + 128 * m : j * COUT + 128 * m + 128],
                        prods[j][:],
                        start=(idx == 0),
                        stop=(idx == 4),
                    )
                zt = zpool.tile([128, FB], BF16, tag="zt")
                nc.scalar.activation(
                    zt[:], ps_z[:], AF.Relu,
                    bias=bn2b[:, m : m + 1], scale=bn2s[:, m : m + 1],
                )
                nc.sync.dma_start(out_d[img, 128 * m : 128 * m + 128, s], zt[:])

            # software pipeline, 2-deep: PE stream per window is
            #   mm1(w) | mm2_m0(w-2) | mm2_m1(w-2)
            # so BN1(w) (ACT) and shuffles+products(w) (DVE/GP) hide behind
            # mm2 of older windows and the PE matmuls stay back-to-back.
            pipe = []
            for img in range(IMG_PER_CORE):
                for win in range(NWIN):
                    prods = stage_a(img, win)
                    if len(pipe) == 2:
                        stage_b_m(*pipe[0], 0)
                        stage_b_m(*pipe.pop(0), 1)
                    pipe.append((img, win, prods))
            for ent in pipe:
                stage_b_m(*ent, 0)
                stage_b_m(*ent, 1)

    _split_multi_waits(nc)
    return nc


_cached = {}


def kernel(**inputs):
    x = np.ascontiguousarray(np.asarray(inputs["x"], np.float32))
    args = [
        np.asarray(inputs[k], np.float32)
        for k in ("w1", "b1", "g1", "be1", "m1", "v1", "w2", "b2", "g2", "be2", "m2", "v2")
    ]
    w1t, bn1s, bn1b, w2p, bn2s, bn2b = _host_weights(*args)

    import ml_dtypes
    w2p = w2p.astype(ml_dtypes.bfloat16)
    w1t = w1t.astype(ml_dtypes.bfloat16)
    if "nc" not in _cached:
        _cached["nc"] = _build_nc()
    nc = _cached["nc"]

    xr = x.reshape(B, CIN, NPIX).astype(ml_dtypes.bfloat16)
    shared = {
        "w1t": w1t, "bn1s": bn1s, "bn1b": bn1b,
        "w2p": w2p, "bn2s": bn2s, "bn2b": bn2b,
    }
    in_maps = [
        {"x": np.ascontiguousarray(xr[c * IMG_PER_CORE : (c + 1) * IMG_PER_CORE]), **shared}
        for c in range(N_CORES)
    ]
    res = run_bass_kernel_spmd(nc, in_maps, core_ids=list(range(N_CORES)))
    kernel.last_results = res
    out = np.concatenate(
        [np.asarray(res.results[c]["out"]) for c in range(N_CORES)], axis=0
    ).astype(np.float32)
    return out.reshape(B, COUT, H, W)
